# revision 38
# baseline (speedup 1.0000x reference)
"""EquivariantLayer GNN message passing on 8 Trainium2 NeuronCores.

v3 strategy (node-parallel, folded weights, host-computed rel):
- Host computes rel = pos[row]-pos[col] per edge, ships it bf16 in a
  three-tier degree-padded layout [P, 3, W] plus per-slot 1/count;
  all on-device math runs in f32 (rel's one bf16 rounding is the only
  low-precision step, ~0.2% relative).
- Scores collapse to a quadratic form in rel (6 monomials x 3 head
  deltas vs head 3, softmax shift-invariance drops head 3's exp);
  monomial squares on Activation, crosses + 18-term contraction on
  DVE split in two column regions so exp/denominator/features of
  region 1 pipeline against region 2's scores.
- Per-slot 1/count and 1/softmax-denominator fold into the edge
  features; slot sums use in-place pairwise tree adds.
- PE transpose + fp32r matmul contract 16 feature channels with the
  folded Wv@Wout. G's columns are centered on the host, folding the
  LayerNorm mean subtraction into the matmul (var = sumsq/32 exact).
- LayerNorm runs channel-major in 4 node-chunks: variance pass for
  all chunks first, then normalize+SiLU+store per chunk (2 act-table
  loads total instead of 8), contiguous per-chunk stores.
"""
import numpy as np
import ml_dtypes

N_NODES = 100000
N_EDGES = 500000
HIDDEN = 32
HEADS = 4
LN_EPS = 1e-5
N_CORES = 8

P = 128
NPC = N_NODES // N_CORES          # 12500 nodes per core
TIERS = [(4, 45), (8, 50), (18, 8)]   # (max degree, node-locs/partition)
T_D = [t[0] for t in TIERS]
T_LOC = [t[1] for t in TIERS]
T_W = [d * l for d, l in TIERS]       # 180 / 400 / 144
T_W0 = [0, T_W[0], T_W[0] + T_W[1]]
T_L0 = [0, T_LOC[0], T_LOC[0] + T_LOC[1]]
W = sum(T_W)                      # 724
NL = sum(T_LOC)                   # 103
NLP = 104                         # 13 transpose blocks of 8
WD = 428                          # region split (tier1 node-aligned)
NCHUNK = 4


def _fold_weights(Wq, bq, Wk, bk, Wv, bv, Wout):
    s = 1.0 / np.sqrt(np.float32(HIDDEN))
    C = np.zeros((10, HEADS), np.float32)
    Gaug = np.zeros((16, 32), np.float32)
    D = HIDDEN
    for h in range(HEADS):
        Wqh, Wkh = Wq[:, h * D:(h + 1) * D], Wk[:, h * D:(h + 1) * D]
        bqh, bkh = bq[h * D:(h + 1) * D], bk[h * D:(h + 1) * D]
        A = (Wqh @ Wkh.T) * s
        # monomial order: xx yy zz xy xz yz
        C[0, h] = A[0, 0]; C[1, h] = A[1, 1]; C[2, h] = A[2, 2]
        C[3, h] = A[0, 1] + A[1, 0]; C[4, h] = A[0, 2] + A[2, 0]
        C[5, h] = A[1, 2] + A[2, 1]
        C[6:9, h] = (Wqh @ bkh + Wkh @ bqh) * s
        C[9, h] = np.dot(bqh, bkh) * s
        Wvh, bvh = Wv[:, h * D:(h + 1) * D], bv[h * D:(h + 1) * D]
        Wouth = Wout[h * D:(h + 1) * D, :]
        Gh = Wvh @ Wouth
        for d in range(3):
            Gaug[3 * h + d, :] = Gh[d]
        Gaug[12 + h, :] = bvh @ Wouth
    # head-delta coefficients: u_h = s_h - s_3 for h = 0..2
    Cu = C[:, 0:3] - C[:, 3:4]            # [10, 3]
    return C, Cu, Gaug


def _build_bass(Cu, use_bout, use_affine, use_gbias, use_lin):
    import concourse.bass as bass
    import concourse.bacc as bacc
    import concourse.mybir as mybir
    import concourse.tile as tile
    from concourse.masks import make_identity

    f32 = mybir.dt.float32
    bf = mybir.dt.bfloat16
    f32r = mybir.dt.float32r
    Alu = mybir.AluOpType
    Act = mybir.ActivationFunctionType

    nc = bacc.Bacc("TRN2", target_bir_lowering=False, debug=False,
                   num_devices=N_CORES)
    REL_in = nc.dram_tensor("REL", [P, 3, W], bf, kind="ExternalInput").ap()
    RCFS_in = nc.dram_tensor("RCFS", [P, W], bf, kind="ExternalInput").ap()
    G_in = nc.dram_tensor("G", [P, 256], f32r, kind="ExternalInput").ap()
    AUX_in = nc.dram_tensor("AUX", [P, 3, 32], f32, kind="ExternalInput").ap()
    y = nc.dram_tensor("y", [P, 32 * NLP], f32, kind="ExternalOutput").ap()

    # all vector work on DVE (TensorScalarPtr is illegal on Pool)
    regions = [(0, WD), (WD, W)]

    with tile.TileContext(nc) as tc:
        with (
            tc.tile_pool(name="sbuf", bufs=1) as sb,
            tc.tile_pool(name="psum", bufs=4, space="PSUM") as ps,
        ):
            REL = sb.tile([P, 3, W], bf, name="REL")
            RCFS = sb.tile([P, W], bf, name="RCFS")
            G = sb.tile([P, 256], f32r, name="G")
            AUX = sb.tile([P, 3, 32], f32, name="AUX")
            for (w0, w1) in regions:
                nc.sync.dma_start(out=REL[:, :, w0:w1],
                                  in_=REL_in[:, :, w0:w1])
            nc.sync.dma_start(out=RCFS[:], in_=RCFS_in[:])
            nc.sync.dma_start(out=G[:], in_=G_in[:])
            nc.sync.dma_start(out=AUX[:], in_=AUX_in[:])

            # ---- monomials M6 = [xx, yy, zz, xy, xz, yz] (f32) ----
            M6 = sb.tile([P, 6, W], f32, name="M6")
            for ri, (w0, w1) in enumerate(regions):
                # region-2 crosses on Pool (TT mult is gpsimd-implemented)
                e = nc.vector if ri == 0 else nc.gpsimd
                nc.scalar.activation(out=M6[:, 0:3, w0:w1],
                                     in_=REL[:, :, w0:w1], func=Act.Square)
                e.tensor_tensor(
                    out=M6[:, 3:5, w0:w1],
                    in0=REL[:, 0:1, w0:w1].broadcast_to([P, 2, w1 - w0]),
                    in1=REL[:, 1:3, w0:w1], op=Alu.mult)
                e.tensor_tensor(out=M6[:, 5, w0:w1],
                                in0=REL[:, 1, w0:w1],
                                in1=REL[:, 2, w0:w1], op=Alu.mult)

            # ---- per-region score pipelines (separate tiles: exact deps) --
            rtiles = {}
            for ri, (w0, w1) in enumerate(regions):
                ww = w1 - w0
                U = sb.tile([P, 3, ww], f32, name=f"U{ri}")
                for h in range(3):
                    c9 = float(Cu[9, h]) if use_lin else 0.0
                    nc.vector.tensor_scalar(
                        out=U[:, h, :], in0=M6[:, 0, w0:w1],
                        scalar1=float(Cu[0, h]), scalar2=c9,
                        op0=Alu.mult, op1=Alu.add)
                    for k in range(1, 6):
                        nc.vector.scalar_tensor_tensor(
                            out=U[:, h, :], in0=M6[:, k, w0:w1],
                            scalar=float(Cu[k, h]), in1=U[:, h, :],
                            op0=Alu.mult, op1=Alu.add)
                    if use_lin:
                        for d in range(3):
                            nc.vector.scalar_tensor_tensor(
                                out=U[:, h, :], in0=REL[:, d, w0:w1],
                                scalar=float(Cu[6 + d, h]), in1=U[:, h, :],
                                op0=Alu.mult, op1=Alu.add)
                # exp + denominator D = 1 + e0 + e1 + e2 ; srn = rcfs / D
                E = sb.tile([P, 3, ww], f32, name=f"E{ri}")
                srn = sb.tile([P, ww], f32, name=f"srn{ri}")
                t1 = sb.tile([P, ww], f32, name=f"t1_{ri}")
                t2 = sb.tile([P, ww], f32, name=f"t2_{ri}")
                nc.scalar.activation(out=E[:], in_=U[:], func=Act.Exp)
                nc.vector.tensor_tensor(out=t1[:], in0=E[:, 0, :],
                                        in1=E[:, 1, :], op=Alu.add)
                nc.vector.tensor_scalar(out=t2[:], in0=E[:, 2, :],
                                        scalar1=1.0, scalar2=None,
                                        op0=Alu.add)
                nc.vector.tensor_tensor(out=t1[:], in0=t1[:], in1=t2[:],
                                        op=Alu.add)
                nc.vector.reciprocal(out=t2[:], in_=t1[:])
                nc.vector.tensor_tensor(out=srn[:], in0=RCFS[:, w0:w1],
                                        in1=t2[:], op=Alu.mult)
                # rel'' = REL * srn ; F planes 0:9 = E_h * rel''_d
                R2 = sb.tile([P, 3, ww], f32, name=f"R2_{ri}")
                F9 = sb.tile([P, 9, ww], f32, name=f"F9_{ri}")
                nc.vector.tensor_tensor(
                    out=R2[:], in0=REL[:, :, w0:w1],
                    in1=srn[:].unsqueeze(1).broadcast_to([P, 3, ww]),
                    op=Alu.mult)
                nc.vector.tensor_tensor(
                    out=F9[:].rearrange("p (h d) w -> p h d w", d=3),
                    in0=E[:].unsqueeze(2).broadcast_to([P, 3, 3, ww]),
                    in1=R2[:].unsqueeze(1).broadcast_to([P, 3, 3, ww]),
                    op=Alu.mult)
                GB = None
                if use_gbias:
                    GB = sb.tile([P, 4, ww], f32, name=f"GB{ri}")
                    nc.vector.tensor_tensor(
                        out=GB[:, 0:3, :], in0=E[:],
                        in1=srn[:].unsqueeze(1).broadcast_to([P, 3, ww]),
                        op=Alu.mult)
                    nc.vector.tensor_scalar(out=GB[:, 3, :], in0=srn[:],
                                            scalar1=1.0, scalar2=None,
                                            op0=Alu.mult)
                rtiles[ri] = (w0, R2, F9, GB)
            # preload the sqrt act table after both exps (off crit path;
            # Copy and Square live in every table, so the only on-path
            # reload left is silu's, once)
            warm = sb.tile([P, 1], f32, name="warm")
            nc.vector.memset(warm[:], 0.5)
            nc.scalar.activation(out=warm[:], in_=warm[:], func=Act.Sqrt)

            # ---- slot reduction: in-place pairwise tree per segment ----
            Fagg = sb.tile([P, NLP, 16], f32, name="Fagg")
            if not use_gbias:
                nc.vector.memset(Fagg[:, :, 12:16], 0.0)
            nc.vector.memset(Fagg[:, NL:, :], 0.0)
            # segments: (d, loc0, nloc, region, local_w0); tier1 split at
            # the region boundary so region-1 trees start early
            n1a = (WD - T_W[0]) // T_D[1]
            segs = [
                (T_D[0], T_L0[0], T_LOC[0], 0, T_W0[0], "vector"),
                (T_D[1], T_L0[1], n1a, 0, T_W0[1], "vector"),
                (T_D[1], T_L0[1] + n1a, T_LOC[1] - n1a, 1, 0, "vector"),
                (T_D[2], T_L0[2], T_LOC[2], 1, T_W0[2] - WD, "gpsimd"),
            ]
            for (d, l0, l, ri, w0, eng) in segs:
                e = getattr(nc, eng)
                _, R2r, F9r, GBr = rtiles[ri]
                srcs = [(F9r, 9, 0), (R2r, 3, 9)]
                if use_gbias:
                    srcs.append((GBr, 4, 12))
                for (src, nch, c0) in srcs:
                    s = d
                    v = src[:, :, w0:w0 + d * l].rearrange(
                        "p c (n s) -> p c n s", s=d)
                    while s > 2:
                        k = s // 2
                        e.tensor_tensor(
                            out=v[:, :, :, 0:k], in0=v[:, :, :, 0:k],
                            in1=v[:, :, :, k:2 * k], op=Alu.add)
                        if s % 2:
                            e.tensor_tensor(
                                out=v[:, :, :, 0], in0=v[:, :, :, 0],
                                in1=v[:, :, :, s - 1], op=Alu.add)
                        s = k
                    out_ap = Fagg[:, l0:l0 + l, c0:c0 + nch].rearrange(
                        "p n c -> p c n")
                    if s == 2:
                        e.tensor_tensor(
                            out=out_ap, in0=v[:, :, :, 0],
                            in1=v[:, :, :, 1], op=Alu.add)
                    else:
                        nc.vector.tensor_scalar(
                            out=out_ap, in0=v[:, :, :, 0],
                            scalar1=1.0, scalar2=None, op0=Alu.mult)

            # ---- PE: transpose 8-loc blocks, contract with centered G ----
            ident = sb.tile([P, P], f32, name="ident")
            make_identity(nc, ident[:])
            Xb = sb.tile([P, 32, NLP], f32, name="Xb")   # channel-major
            for b in range(NLP // 8):
                tps = ps.tile([P, P], f32, space="PSUM", tag="tps")
                nc.tensor.transpose(
                    out=tps[:],
                    in_=Fagg[:, 8 * b:8 * b + 8, :].rearrange(
                        "p a j -> p (a j)"),
                    identity=ident[:])
                tsb = sb.tile([P, P], f32r, name=f"tsb{b % 3}",
                              tag=f"tsb{b % 3}")
                nc.scalar.activation(out=tsb[:], in_=tps[:], func=Act.Copy)
                seg_ps = ps.tile([P, 256], f32, space="PSUM", tag="seg")
                nc.tensor.matmul(out=seg_ps[:], lhsT=tsb[:], rhs=G[:],
                                 start=True, stop=True)
                nc.scalar.activation(
                    out=Xb[:, :, 8 * b:8 * b + 8].rearrange("p c a -> p a c"),
                    in_=seg_ps[:].rearrange("p (a c) -> p a c", c=32),
                    func=Act.Copy)

            # ---- LayerNorm (channel-major, f32), 4 node-chunks ----
            # G is column-centered so Xb is already x - mu and
            # var = sum(Xb^2)/32 exactly. Pass 1: variance + rstd for all
            # chunks (one sqrt-table load); pass 2: normalize+silu+store
            # (one silu-table load).
            CH = NLP // NCHUNK
            XS = sb.tile([P, 32, NLP], f32, name="XS")
            var = sb.tile([P, NLP], f32, name="var")
            rv = sb.tile([P, NLP], f32, name="rv")
            rstd = sb.tile([P, NLP], f32, name="rstd")
            for ci in range(NCHUNK):
                n0, n1 = ci * CH, (ci + 1) * CH
                if use_bout:
                    # host ships bout - mean(bout); mean stays zero
                    nc.vector.tensor_tensor(
                        out=Xb[:, :, n0:n1], in0=Xb[:, :, n0:n1],
                        in1=AUX[:, 0, :].rearrange("p c -> p c 1")
                        .broadcast_to([P, 32, CH]), op=Alu.add)
                nc.scalar.activation(out=XS[:, :, n0:n1],
                                     in_=Xb[:, :, n0:n1], func=Act.Square)
                c = 32
                while c > 1:
                    k = c // 2
                    nc.vector.tensor_tensor(out=XS[:, 0:k, n0:n1],
                                            in0=XS[:, 0:k, n0:n1],
                                            in1=XS[:, k:c, n0:n1],
                                            op=Alu.add)
                    c = k
                nc.vector.tensor_scalar(
                    out=var[:, n0:n1], in0=XS[:, 0, n0:n1],
                    scalar1=1.0 / 32, scalar2=LN_EPS,
                    op0=Alu.mult, op1=Alu.add)
                nc.vector.reciprocal(out=rv[:, n0:n1], in_=var[:, n0:n1])
                nc.scalar.activation(out=rstd[:, n0:n1], in_=rv[:, n0:n1],
                                     func=Act.Sqrt)
            for ci in range(NCHUNK):
                n0, n1 = ci * CH, (ci + 1) * CH
                XM = sb.tile([P, 32, CH], f32, name=f"XM{ci}",
                             tag=f"XM{ci}")
                nc.vector.tensor_tensor(
                    out=XM[:], in0=Xb[:, :, n0:n1],
                    in1=rstd[:, n0:n1].unsqueeze(1).broadcast_to(
                        [P, 32, CH]), op=Alu.mult)
                if use_affine:
                    nc.vector.tensor_tensor(
                        out=XM[:], in0=XM[:],
                        in1=AUX[:, 1, :].rearrange("p c -> p c 1")
                        .broadcast_to([P, 32, CH]), op=Alu.mult)
                    nc.vector.tensor_tensor(
                        out=XM[:], in0=XM[:],
                        in1=AUX[:, 2, :].rearrange("p c -> p c 1")
                        .broadcast_to([P, 32, CH]), op=Alu.add)
                OUTc = sb.tile([P, 32, CH], f32, name=f"OUT{ci}",
                               tag=f"OUT{ci}")
                nc.scalar.activation(out=OUTc[:], in_=XM[:], func=Act.Silu)
                nc.sync.dma_start(
                    out=y[:].rearrange("p (ci c n) -> p ci c n", ci=NCHUNK,
                                       c=32)[:, ci, :, :],
                    in_=OUTc[:])
    nc.compile()
    return nc


_CACHE = {}


def _prep(positions, edge_index):
    pos = np.asarray(positions, np.float32)
    row = np.asarray(edge_index[0], np.int64)
    col = np.asarray(edge_index[1], np.int64)
    deg = np.bincount(col, minlength=N_NODES)
    assert deg.max() <= T_D[2], f"max degree {deg.max()} exceeds {T_D[2]}"
    order = np.argsort(col, kind="stable")
    col_s, row_s = col[order], row[order]
    starts = np.zeros(N_NODES + 1, np.int64)
    np.cumsum(deg, out=starts[1:])
    rel_all = (pos[row_s] - pos[col_s]).astype(np.float32)

    in_maps, metas = [], []
    for c in range(N_CORES):
        base = c * NPC
        dloc = deg[base:base + NPC]
        tier = np.where(dloc <= T_D[0], 0, np.where(dloc <= T_D[1], 1, 2))
        REL = np.zeros((P, 3, W), np.float32)
        RCFS = np.zeros((P, W), np.float32)
        k_of = np.zeros(NPC, np.int64)
        rows_of = np.zeros(NPC, np.int64)
        for ti in range(3):
            ids = np.flatnonzero(tier == ti)
            cap = T_LOC[ti] * P
            assert len(ids) <= cap, f"tier {ti}: {len(ids)} > {cap}"
            k = np.arange(len(ids))
            k_of[ids] = k
            pp, ll = k // T_LOC[ti], k % T_LOC[ti]
            rows_of[ids] = pp * NLP + T_L0[ti] + ll
        e0, e1 = starts[base], starts[base + NPC]
        n_loc = (col_s[e0:e1] - base).astype(np.int64)
        slot = np.arange(e0, e1) - starts[col_s[e0:e1]]
        for ti in range(3):
            m = tier[n_loc] == ti
            k = k_of[n_loc[m]]
            pp = k // T_LOC[ti]
            ww = T_W0[ti] + (k % T_LOC[ti]) * T_D[ti] + slot[m]
            REL[pp, :, ww] = rel_all[e0:e1][m]
            RCFS[pp, ww] = 1.0 / dloc[n_loc[m]]
        in_maps.append({
            "REL": REL.astype(ml_dtypes.bfloat16),
            "RCFS": RCFS.astype(ml_dtypes.bfloat16),
        })
        metas.append(rows_of)
    return in_maps, metas


_EXEC = {}


def _run_cached(nc, in_maps):
    import jax
    import numpy as _np
    import concourse.mybir as mybir
    from jax.sharding import Mesh, PartitionSpec
    from jax.experimental.shard_map import shard_map
    from concourse import bass2jax as B2J

    key = id(nc)
    if key not in _EXEC:
        B2J.install_neuronx_cc_hook()
        partition_name = (nc.partition_id_tensor.name
                          if nc.partition_id_tensor else None)
        in_names, out_names, out_avals, zero_shapes = [], [], [], []
        for alloc in nc.m.functions[0].allocations:
            if not isinstance(alloc, mybir.MemoryLocationSet):
                continue
            name = alloc.memorylocations[0].name
            if alloc.kind == "ExternalInput":
                if name != partition_name:
                    in_names.append(name)
            elif alloc.kind == "ExternalOutput":
                out_names.append(name)
                shape = tuple(alloc.tensor_shape)
                dtype = mybir.dt.np(alloc.dtype)
                out_avals.append(jax.core.ShapedArray(shape, dtype))
                zero_shapes.append((shape, dtype))
        n_params = len(in_names)
        all_in = list(in_names) + list(out_names)
        if partition_name is not None:
            all_in.append(partition_name)
        donate = tuple(range(n_params, n_params + len(out_names)))

        def _body(*args):
            operands = list(args)
            if partition_name is not None:
                operands.append(B2J.partition_id_tensor())
            return tuple(B2J._bass_exec_p.bind(
                *operands, out_avals=tuple(out_avals), in_names=tuple(all_in),
                out_names=tuple(out_names), lowering_input_output_aliases=(),
                sim_require_finite=True, sim_require_nnan=True, nc=nc))

        devices = jax.devices()[:N_CORES]
        mesh = Mesh(_np.asarray(devices), ("core",))
        specs = (PartitionSpec("core"),) * (n_params + len(out_names))
        fn = jax.jit(
            shard_map(_body, mesh=mesh, in_specs=specs,
                      out_specs=(PartitionSpec("core"),) * len(out_names),
                      check_rep=False),
            donate_argnums=donate, keep_unused=True)
        _EXEC[key] = (fn, in_names, out_names, out_avals, zero_shapes)

    fn, in_names, out_names, out_avals, zero_shapes = _EXEC[key]
    concat_in = [np.concatenate([np.asarray(m[name]) for m in in_maps], axis=0)
                 for name in in_names]
    zeros = [np.zeros((N_CORES * s[0], *s[1:]), d) for s, d in zero_shapes]
    outs = fn(*concat_in, *zeros)
    return [
        {name: np.asarray(outs[i]).reshape(N_CORES, *out_avals[i].shape)[c]
         for i, name in enumerate(out_names)}
        for c in range(N_CORES)
    ]


def kernel(positions, edge_index, Wq, bq, Wk, bk, Wv, bv, Wout, bout,
           gamma, beta):
    positions = np.asarray(positions, np.float32)
    args = [np.asarray(x, np.float32)
            for x in (Wq, bq, Wk, bk, Wv, bv, Wout)]
    bout = np.asarray(bout, np.float32)
    gamma = np.asarray(gamma, np.float32)
    beta = np.asarray(beta, np.float32)
    C, Cu, Gaug = _fold_weights(*args)
    use_bout = bool(np.any(bout != 0))
    use_affine = bool(np.any(gamma != 1) or np.any(beta != 0))
    use_gbias = bool(np.any(Gaug[12:16, :] != 0))
    use_lin = bool(np.any(C[6:10, :] != 0))

    key = (use_bout, use_affine, use_gbias, use_lin)
    if key not in _CACHE:
        _CACHE[key] = _build_bass(Cu, use_bout, use_affine, use_gbias,
                                  use_lin)
    nc = _CACHE[key]

    in_maps, metas = _prep(positions, edge_index)
    # block-diagonal, column-centered Gaug for the 8-loc transpose blocks:
    # centering folds the LayerNorm mean subtraction into the matmul
    Gc = Gaug - Gaug.mean(axis=1, keepdims=True)
    Gblk = np.zeros((P, 256), np.float32)
    for loc in range(8):
        Gblk[16 * loc:16 * loc + 16, 32 * loc:32 * loc + 32] = Gc
    for m in in_maps:
        m["G"] = Gblk
        aux = np.zeros((P, 3, 32), np.float32)
        aux[:, 0, :] = bout - bout.mean()
        aux[:, 1, :] = gamma
        aux[:, 2, :] = beta
        m["AUX"] = aux
    res = _run_cached(nc, in_maps)

    out = np.empty((N_NODES, 32), np.float32)
    for c in range(N_CORES):
        base = c * NPC
        yv = res[c]["y"].reshape(P, NCHUNK, 32, NLP // NCHUNK)
        flat = np.ascontiguousarray(np.transpose(yv, (0, 1, 3, 2))).reshape(
            P * NLP, 32)
        out[base:base + NPC] = flat[metas[c]]
    return out


# revision 39
# speedup vs baseline: 1.0084x; 1.0084x over previous
"""EquivariantLayer GNN message passing on 8 Trainium2 NeuronCores.

v3 strategy (node-parallel, folded weights, host-computed rel):
- Host computes rel = pos[row]-pos[col] per edge, ships it bf16 in a
  three-tier degree-padded layout [P, 3, W] plus per-slot 1/count;
  all on-device math runs in f32 (rel's one bf16 rounding is the only
  low-precision step, ~0.2% relative).
- Scores collapse to a quadratic form in rel (6 monomials x 3 head
  deltas vs head 3, softmax shift-invariance drops head 3's exp);
  monomial squares on Activation, crosses + 18-term contraction on
  DVE split in two column regions so exp/denominator/features of
  region 1 pipeline against region 2's scores.
- Per-slot 1/count and 1/softmax-denominator fold into the edge
  features; slot sums use in-place pairwise tree adds.
- PE transpose + fp32r matmul contract 16 feature channels with the
  folded Wv@Wout. G's columns are centered on the host, folding the
  LayerNorm mean subtraction into the matmul (var = sumsq/32 exact).
- LayerNorm runs channel-major in 4 node-chunks: variance pass for
  all chunks first, then normalize+SiLU+store per chunk (2 act-table
  loads total instead of 8), contiguous per-chunk stores.
"""
import numpy as np
import ml_dtypes

N_NODES = 100000
N_EDGES = 500000
HIDDEN = 32
HEADS = 4
LN_EPS = 1e-5
N_CORES = 8

P = 128
NPC = N_NODES // N_CORES          # 12500 nodes per core
TIERS = [(4, 45), (8, 50), (18, 8)]   # (max degree, node-locs/partition)
T_D = [t[0] for t in TIERS]
T_LOC = [t[1] for t in TIERS]
T_W = [d * l for d, l in TIERS]       # 180 / 400 / 144
T_W0 = [0, T_W[0], T_W[0] + T_W[1]]
T_L0 = [0, T_LOC[0], T_LOC[0] + T_LOC[1]]
W = sum(T_W)                      # 724
NL = sum(T_LOC)                   # 103
NLP = 104                         # 13 transpose blocks of 8
WD = 428                          # region split (tier1 node-aligned)
NCHUNK = 4


def _fold_weights(Wq, bq, Wk, bk, Wv, bv, Wout):
    s = 1.0 / np.sqrt(np.float32(HIDDEN))
    C = np.zeros((10, HEADS), np.float32)
    Gaug = np.zeros((16, 32), np.float32)
    D = HIDDEN
    for h in range(HEADS):
        Wqh, Wkh = Wq[:, h * D:(h + 1) * D], Wk[:, h * D:(h + 1) * D]
        bqh, bkh = bq[h * D:(h + 1) * D], bk[h * D:(h + 1) * D]
        A = (Wqh @ Wkh.T) * s
        # monomial order: xx yy zz xy xz yz
        C[0, h] = A[0, 0]; C[1, h] = A[1, 1]; C[2, h] = A[2, 2]
        C[3, h] = A[0, 1] + A[1, 0]; C[4, h] = A[0, 2] + A[2, 0]
        C[5, h] = A[1, 2] + A[2, 1]
        C[6:9, h] = (Wqh @ bkh + Wkh @ bqh) * s
        C[9, h] = np.dot(bqh, bkh) * s
        Wvh, bvh = Wv[:, h * D:(h + 1) * D], bv[h * D:(h + 1) * D]
        Wouth = Wout[h * D:(h + 1) * D, :]
        Gh = Wvh @ Wouth
        for d in range(3):
            Gaug[3 * h + d, :] = Gh[d]
        Gaug[12 + h, :] = bvh @ Wouth
    # head-delta coefficients: u_h = s_h - s_3 for h = 0..2
    Cu = C[:, 0:3] - C[:, 3:4]            # [10, 3]
    return C, Cu, Gaug


def _build_bass(Cu, use_bout, use_affine, use_gbias, use_lin):
    import concourse.bass as bass
    import concourse.bacc as bacc
    import concourse.mybir as mybir
    import concourse.tile as tile
    from concourse.masks import make_identity

    f32 = mybir.dt.float32
    bf = mybir.dt.bfloat16
    f32r = mybir.dt.float32r
    Alu = mybir.AluOpType
    Act = mybir.ActivationFunctionType

    nc = bacc.Bacc("TRN2", target_bir_lowering=False, debug=False,
                   num_devices=N_CORES)
    REL_in = nc.dram_tensor("REL", [P, 3, W], bf, kind="ExternalInput").ap()
    RCFS_in = nc.dram_tensor("RCFS", [P, W], bf, kind="ExternalInput").ap()
    G_in = nc.dram_tensor("G", [P, 256], f32r, kind="ExternalInput").ap()
    AUX_in = nc.dram_tensor("AUX", [P, 3, 32], f32, kind="ExternalInput").ap()
    y = nc.dram_tensor("y", [P, 32 * NLP], f32, kind="ExternalOutput").ap()

    # all vector work on DVE (TensorScalarPtr is illegal on Pool)
    regions = [(0, WD), (WD, W)]

    with tile.TileContext(nc) as tc:
        with (
            tc.tile_pool(name="sbuf", bufs=1) as sb,
            tc.tile_pool(name="psum", bufs=4, space="PSUM") as ps,
        ):
            REL = sb.tile([P, 3, W], bf, name="REL")
            RCFS = sb.tile([P, W], bf, name="RCFS")
            G = sb.tile([P, 256], f32r, name="G")
            AUX = sb.tile([P, 3, 32], f32, name="AUX")
            for (w0, w1) in regions:
                nc.sync.dma_start(out=REL[:, :, w0:w1],
                                  in_=REL_in[:, :, w0:w1])
            nc.sync.dma_start(out=RCFS[:], in_=RCFS_in[:])
            nc.sync.dma_start(out=G[:], in_=G_in[:])
            nc.sync.dma_start(out=AUX[:], in_=AUX_in[:])

            # ---- monomials M6 = [xx, yy, zz, xy, xz, yz] (f32) ----
            M6 = sb.tile([P, 6, W], f32, name="M6")
            for (w0, w1) in regions:
                nc.scalar.activation(out=M6[:, 0:3, w0:w1],
                                     in_=REL[:, :, w0:w1], func=Act.Square)
                nc.vector.tensor_tensor(
                    out=M6[:, 3:5, w0:w1],
                    in0=REL[:, 0:1, w0:w1].broadcast_to([P, 2, w1 - w0]),
                    in1=REL[:, 1:3, w0:w1], op=Alu.mult)
                nc.vector.tensor_tensor(out=M6[:, 5, w0:w1],
                                        in0=REL[:, 1, w0:w1],
                                        in1=REL[:, 2, w0:w1], op=Alu.mult)

            # ---- per-region score pipelines (separate tiles: exact deps) --
            rtiles = {}
            for ri, (w0, w1) in enumerate(regions):
                ww = w1 - w0
                U = sb.tile([P, 3, ww], f32, name=f"U{ri}")
                for h in range(3):
                    c9 = float(Cu[9, h]) if use_lin else 0.0
                    nc.vector.tensor_scalar(
                        out=U[:, h, :], in0=M6[:, 0, w0:w1],
                        scalar1=float(Cu[0, h]), scalar2=c9,
                        op0=Alu.mult, op1=Alu.add)
                    for k in range(1, 6):
                        nc.vector.scalar_tensor_tensor(
                            out=U[:, h, :], in0=M6[:, k, w0:w1],
                            scalar=float(Cu[k, h]), in1=U[:, h, :],
                            op0=Alu.mult, op1=Alu.add)
                    if use_lin:
                        for d in range(3):
                            nc.vector.scalar_tensor_tensor(
                                out=U[:, h, :], in0=REL[:, d, w0:w1],
                                scalar=float(Cu[6 + d, h]), in1=U[:, h, :],
                                op0=Alu.mult, op1=Alu.add)
                # exp + denominator D = 1 + e0 + e1 + e2 ; srn = rcfs / D
                E = sb.tile([P, 3, ww], f32, name=f"E{ri}")
                srn = sb.tile([P, ww], f32, name=f"srn{ri}")
                t1 = sb.tile([P, ww], f32, name=f"t1_{ri}")
                t2 = sb.tile([P, ww], f32, name=f"t2_{ri}")
                nc.scalar.activation(out=E[:], in_=U[:], func=Act.Exp)
                nc.vector.tensor_tensor(out=t1[:], in0=E[:, 0, :],
                                        in1=E[:, 1, :], op=Alu.add)
                nc.vector.tensor_scalar(out=t2[:], in0=E[:, 2, :],
                                        scalar1=1.0, scalar2=None,
                                        op0=Alu.add)
                nc.vector.tensor_tensor(out=t1[:], in0=t1[:], in1=t2[:],
                                        op=Alu.add)
                nc.vector.reciprocal(out=t2[:], in_=t1[:])
                nc.vector.tensor_tensor(out=srn[:], in0=RCFS[:, w0:w1],
                                        in1=t2[:], op=Alu.mult)
                # rel'' = REL * srn ; F planes 0:9 = E_h * rel''_d
                R2 = sb.tile([P, 3, ww], f32, name=f"R2_{ri}")
                F9 = sb.tile([P, 9, ww], f32, name=f"F9_{ri}")
                nc.vector.tensor_tensor(
                    out=R2[:], in0=REL[:, :, w0:w1],
                    in1=srn[:].unsqueeze(1).broadcast_to([P, 3, ww]),
                    op=Alu.mult)
                nc.vector.tensor_tensor(
                    out=F9[:].rearrange("p (h d) w -> p h d w", d=3),
                    in0=E[:].unsqueeze(2).broadcast_to([P, 3, 3, ww]),
                    in1=R2[:].unsqueeze(1).broadcast_to([P, 3, 3, ww]),
                    op=Alu.mult)
                GB = None
                if use_gbias:
                    GB = sb.tile([P, 4, ww], f32, name=f"GB{ri}")
                    nc.vector.tensor_tensor(
                        out=GB[:, 0:3, :], in0=E[:],
                        in1=srn[:].unsqueeze(1).broadcast_to([P, 3, ww]),
                        op=Alu.mult)
                    nc.vector.tensor_scalar(out=GB[:, 3, :], in0=srn[:],
                                            scalar1=1.0, scalar2=None,
                                            op0=Alu.mult)
                rtiles[ri] = (w0, R2, F9, GB)
            # preload the sqrt act table after both exps (off crit path;
            # Copy and Square live in every table, so the only on-path
            # reload left is silu's, once)
            warm = sb.tile([P, 1], f32, name="warm")
            nc.vector.memset(warm[:], 0.5)
            nc.scalar.activation(out=warm[:], in_=warm[:], func=Act.Sqrt)

            # ---- slot reduction: in-place pairwise tree per segment ----
            Fagg = sb.tile([P, NLP, 16], f32, name="Fagg")
            if not use_gbias:
                nc.vector.memset(Fagg[:, :, 12:16], 0.0)
            nc.vector.memset(Fagg[:, NL:, :], 0.0)
            # segments: (d, loc0, nloc, region, local_w0); tier1 split at
            # the region boundary so region-1 trees start early
            n1a = (WD - T_W[0]) // T_D[1]
            segs = [
                (T_D[0], T_L0[0], T_LOC[0], 0, T_W0[0]),
                (T_D[1], T_L0[1], n1a, 0, T_W0[1]),
                (T_D[1], T_L0[1] + n1a, T_LOC[1] - n1a, 1, 0),
                (T_D[2], T_L0[2], T_LOC[2], 1, T_W0[2] - WD),
            ]
            for (d, l0, l, ri, w0) in segs:
                _, R2r, F9r, GBr = rtiles[ri]
                srcs = [(F9r, 9, 0), (R2r, 3, 9)]
                if use_gbias:
                    srcs.append((GBr, 4, 12))
                for (src, nch, c0) in srcs:
                    s = d
                    v = src[:, :, w0:w0 + d * l].rearrange(
                        "p c (n s) -> p c n s", s=d)
                    while s > 2:
                        k = s // 2
                        nc.vector.tensor_tensor(
                            out=v[:, :, :, 0:k], in0=v[:, :, :, 0:k],
                            in1=v[:, :, :, k:2 * k], op=Alu.add)
                        if s % 2:
                            nc.vector.tensor_tensor(
                                out=v[:, :, :, 0], in0=v[:, :, :, 0],
                                in1=v[:, :, :, s - 1], op=Alu.add)
                        s = k
                    out_ap = Fagg[:, l0:l0 + l, c0:c0 + nch].rearrange(
                        "p n c -> p c n")
                    if s == 2:
                        nc.vector.tensor_tensor(
                            out=out_ap, in0=v[:, :, :, 0],
                            in1=v[:, :, :, 1], op=Alu.add)
                    else:
                        nc.vector.tensor_scalar(
                            out=out_ap, in0=v[:, :, :, 0],
                            scalar1=1.0, scalar2=None, op0=Alu.mult)

            # ---- PE: transpose 8-loc blocks, contract with centered G ----
            ident = sb.tile([P, P], f32, name="ident")
            make_identity(nc, ident[:])
            Xb = sb.tile([P, 32, NLP], f32, name="Xb")   # channel-major
            for b in range(NLP // 8):
                tps = ps.tile([P, P], f32, space="PSUM", tag="tps")
                nc.tensor.transpose(
                    out=tps[:],
                    in_=Fagg[:, 8 * b:8 * b + 8, :].rearrange(
                        "p a j -> p (a j)"),
                    identity=ident[:])
                tsb = sb.tile([P, P], f32r, name=f"tsb{b % 3}",
                              tag=f"tsb{b % 3}")
                nc.scalar.activation(out=tsb[:], in_=tps[:], func=Act.Copy)
                seg_ps = ps.tile([P, 256], f32, space="PSUM", tag="seg")
                nc.tensor.matmul(out=seg_ps[:], lhsT=tsb[:], rhs=G[:],
                                 start=True, stop=True)
                nc.scalar.activation(
                    out=Xb[:, :, 8 * b:8 * b + 8].rearrange("p c a -> p a c"),
                    in_=seg_ps[:].rearrange("p (a c) -> p a c", c=32),
                    func=Act.Copy)

            # ---- LayerNorm (channel-major, f32), 4 node-chunks ----
            # G is column-centered so Xb is already x - mu and
            # var = sum(Xb^2)/32 exactly. Pass 1: variance + rstd for all
            # chunks (one sqrt-table load); pass 2: normalize+silu+store
            # (one silu-table load).
            CH = NLP // NCHUNK
            XS = sb.tile([P, 32, NLP], f32, name="XS")
            var = sb.tile([P, NLP], f32, name="var")
            rv = sb.tile([P, NLP], f32, name="rv")
            rstd = sb.tile([P, NLP], f32, name="rstd")
            for ci in range(NCHUNK):
                n0, n1 = ci * CH, (ci + 1) * CH
                if use_bout:
                    # host ships bout - mean(bout); mean stays zero
                    nc.vector.tensor_tensor(
                        out=Xb[:, :, n0:n1], in0=Xb[:, :, n0:n1],
                        in1=AUX[:, 0, :].rearrange("p c -> p c 1")
                        .broadcast_to([P, 32, CH]), op=Alu.add)
                nc.scalar.activation(out=XS[:, :, n0:n1],
                                     in_=Xb[:, :, n0:n1], func=Act.Square)
                c = 32
                while c > 1:
                    k = c // 2
                    nc.vector.tensor_tensor(out=XS[:, 0:k, n0:n1],
                                            in0=XS[:, 0:k, n0:n1],
                                            in1=XS[:, k:c, n0:n1],
                                            op=Alu.add)
                    c = k
                nc.vector.tensor_scalar(
                    out=var[:, n0:n1], in0=XS[:, 0, n0:n1],
                    scalar1=1.0 / 32, scalar2=LN_EPS,
                    op0=Alu.mult, op1=Alu.add)
                nc.vector.reciprocal(out=rv[:, n0:n1], in_=var[:, n0:n1])
                nc.scalar.activation(out=rstd[:, n0:n1], in_=rv[:, n0:n1],
                                     func=Act.Sqrt)
            for ci in range(NCHUNK):
                n0, n1 = ci * CH, (ci + 1) * CH
                XM = sb.tile([P, 32, CH], f32, name=f"XM{ci}",
                             tag=f"XM{ci}")
                nc.vector.tensor_tensor(
                    out=XM[:], in0=Xb[:, :, n0:n1],
                    in1=rstd[:, n0:n1].unsqueeze(1).broadcast_to(
                        [P, 32, CH]), op=Alu.mult)
                if use_affine:
                    nc.vector.tensor_tensor(
                        out=XM[:], in0=XM[:],
                        in1=AUX[:, 1, :].rearrange("p c -> p c 1")
                        .broadcast_to([P, 32, CH]), op=Alu.mult)
                    nc.vector.tensor_tensor(
                        out=XM[:], in0=XM[:],
                        in1=AUX[:, 2, :].rearrange("p c -> p c 1")
                        .broadcast_to([P, 32, CH]), op=Alu.add)
                OUTc = sb.tile([P, 32, CH], f32, name=f"OUT{ci}",
                               tag=f"OUT{ci}")
                nc.scalar.activation(out=OUTc[:], in_=XM[:], func=Act.Silu)
                nc.sync.dma_start(
                    out=y[:].rearrange("p (ci c n) -> p ci c n", ci=NCHUNK,
                                       c=32)[:, ci, :, :],
                    in_=OUTc[:])
    nc.compile()
    return nc


_CACHE = {}


def _prep(positions, edge_index):
    pos = np.asarray(positions, np.float32)
    row = np.asarray(edge_index[0], np.int64)
    col = np.asarray(edge_index[1], np.int64)
    deg = np.bincount(col, minlength=N_NODES)
    assert deg.max() <= T_D[2], f"max degree {deg.max()} exceeds {T_D[2]}"
    order = np.argsort(col, kind="stable")
    col_s, row_s = col[order], row[order]
    starts = np.zeros(N_NODES + 1, np.int64)
    np.cumsum(deg, out=starts[1:])
    rel_all = (pos[row_s] - pos[col_s]).astype(np.float32)

    in_maps, metas = [], []
    for c in range(N_CORES):
        base = c * NPC
        dloc = deg[base:base + NPC]
        tier = np.where(dloc <= T_D[0], 0, np.where(dloc <= T_D[1], 1, 2))
        REL = np.zeros((P, 3, W), np.float32)
        RCFS = np.zeros((P, W), np.float32)
        k_of = np.zeros(NPC, np.int64)
        rows_of = np.zeros(NPC, np.int64)
        for ti in range(3):
            ids = np.flatnonzero(tier == ti)
            cap = T_LOC[ti] * P
            assert len(ids) <= cap, f"tier {ti}: {len(ids)} > {cap}"
            k = np.arange(len(ids))
            k_of[ids] = k
            pp, ll = k // T_LOC[ti], k % T_LOC[ti]
            rows_of[ids] = pp * NLP + T_L0[ti] + ll
        e0, e1 = starts[base], starts[base + NPC]
        n_loc = (col_s[e0:e1] - base).astype(np.int64)
        slot = np.arange(e0, e1) - starts[col_s[e0:e1]]
        for ti in range(3):
            m = tier[n_loc] == ti
            k = k_of[n_loc[m]]
            pp = k // T_LOC[ti]
            ww = T_W0[ti] + (k % T_LOC[ti]) * T_D[ti] + slot[m]
            REL[pp, :, ww] = rel_all[e0:e1][m]
            RCFS[pp, ww] = 1.0 / dloc[n_loc[m]]
        in_maps.append({
            "REL": REL.astype(ml_dtypes.bfloat16),
            "RCFS": RCFS.astype(ml_dtypes.bfloat16),
        })
        metas.append(rows_of)
    return in_maps, metas


_EXEC = {}


def _run_cached(nc, in_maps):
    import jax
    import numpy as _np
    import concourse.mybir as mybir
    from jax.sharding import Mesh, PartitionSpec
    from jax.experimental.shard_map import shard_map
    from concourse import bass2jax as B2J

    key = id(nc)
    if key not in _EXEC:
        B2J.install_neuronx_cc_hook()
        partition_name = (nc.partition_id_tensor.name
                          if nc.partition_id_tensor else None)
        in_names, out_names, out_avals, zero_shapes = [], [], [], []
        for alloc in nc.m.functions[0].allocations:
            if not isinstance(alloc, mybir.MemoryLocationSet):
                continue
            name = alloc.memorylocations[0].name
            if alloc.kind == "ExternalInput":
                if name != partition_name:
                    in_names.append(name)
            elif alloc.kind == "ExternalOutput":
                out_names.append(name)
                shape = tuple(alloc.tensor_shape)
                dtype = mybir.dt.np(alloc.dtype)
                out_avals.append(jax.core.ShapedArray(shape, dtype))
                zero_shapes.append((shape, dtype))
        n_params = len(in_names)
        all_in = list(in_names) + list(out_names)
        if partition_name is not None:
            all_in.append(partition_name)
        donate = tuple(range(n_params, n_params + len(out_names)))

        def _body(*args):
            operands = list(args)
            if partition_name is not None:
                operands.append(B2J.partition_id_tensor())
            return tuple(B2J._bass_exec_p.bind(
                *operands, out_avals=tuple(out_avals), in_names=tuple(all_in),
                out_names=tuple(out_names), lowering_input_output_aliases=(),
                sim_require_finite=True, sim_require_nnan=True, nc=nc))

        devices = jax.devices()[:N_CORES]
        mesh = Mesh(_np.asarray(devices), ("core",))
        specs = (PartitionSpec("core"),) * (n_params + len(out_names))
        fn = jax.jit(
            shard_map(_body, mesh=mesh, in_specs=specs,
                      out_specs=(PartitionSpec("core"),) * len(out_names),
                      check_rep=False),
            donate_argnums=donate, keep_unused=True)
        _EXEC[key] = (fn, in_names, out_names, out_avals, zero_shapes)

    fn, in_names, out_names, out_avals, zero_shapes = _EXEC[key]
    concat_in = [np.concatenate([np.asarray(m[name]) for m in in_maps], axis=0)
                 for name in in_names]
    zeros = [np.zeros((N_CORES * s[0], *s[1:]), d) for s, d in zero_shapes]
    outs = fn(*concat_in, *zeros)
    return [
        {name: np.asarray(outs[i]).reshape(N_CORES, *out_avals[i].shape)[c]
         for i, name in enumerate(out_names)}
        for c in range(N_CORES)
    ]


def kernel(positions, edge_index, Wq, bq, Wk, bk, Wv, bv, Wout, bout,
           gamma, beta):
    positions = np.asarray(positions, np.float32)
    args = [np.asarray(x, np.float32)
            for x in (Wq, bq, Wk, bk, Wv, bv, Wout)]
    bout = np.asarray(bout, np.float32)
    gamma = np.asarray(gamma, np.float32)
    beta = np.asarray(beta, np.float32)
    C, Cu, Gaug = _fold_weights(*args)
    use_bout = bool(np.any(bout != 0))
    use_affine = bool(np.any(gamma != 1) or np.any(beta != 0))
    use_gbias = bool(np.any(Gaug[12:16, :] != 0))
    use_lin = bool(np.any(C[6:10, :] != 0))

    key = (use_bout, use_affine, use_gbias, use_lin)
    if key not in _CACHE:
        _CACHE[key] = _build_bass(Cu, use_bout, use_affine, use_gbias,
                                  use_lin)
    nc = _CACHE[key]

    in_maps, metas = _prep(positions, edge_index)
    # block-diagonal, column-centered Gaug for the 8-loc transpose blocks:
    # centering folds the LayerNorm mean subtraction into the matmul
    Gc = Gaug - Gaug.mean(axis=1, keepdims=True)
    Gblk = np.zeros((P, 256), np.float32)
    for loc in range(8):
        Gblk[16 * loc:16 * loc + 16, 32 * loc:32 * loc + 32] = Gc
    for m in in_maps:
        m["G"] = Gblk
        aux = np.zeros((P, 3, 32), np.float32)
        aux[:, 0, :] = bout - bout.mean()
        aux[:, 1, :] = gamma
        aux[:, 2, :] = beta
        m["AUX"] = aux
    res = _run_cached(nc, in_maps)

    out = np.empty((N_NODES, 32), np.float32)
    for c in range(N_CORES):
        base = c * NPC
        yv = res[c]["y"].reshape(P, NCHUNK, 32, NLP // NCHUNK)
        flat = np.ascontiguousarray(np.transpose(yv, (0, 1, 3, 2))).reshape(
            P * NLP, 32)
        out[base:base + NPC] = flat[metas[c]]
    return out
